# revision 26
# baseline (speedup 1.0000x reference)
"""Causal multi-head attention (B=4, S=2048, D=1024, H=16) on 8 TRN2 cores.

Sharding: core c -> (batch b = c//2, head-group g = c%2, 8 heads each).
Host pre-transposes/splits inputs; device returns per-core partial outputs
y_c = attn_heads(g) @ wo[g-rows] in f16; host sums the two partials per batch.

Single interleaved phase: for each 256-seq-position slab, project q/k/v for
those positions (f32r matmuls, f16 outputs; q pre-scaled by 1/8), then run
attention for the two 128-row q-tiles the slab enables.  QK^T runs as f16
K=64 matmuls straight off the channel-major q/k tiles (measured: K=64 is
full rate, so no k duplication needed).  The causal mask (-60000) folds into
the score PSUM accumulation via an ident@maskw matmul.  Row-max on DVE,
exp on ScalarE (f16 out), P transposed to k-major via xbar DMA batched per
head pair, PV as N=65 f16 matmuls (ones column gives Z), normalize on DVE,
per-qt output projection pipelined one q-tile behind.

Pipelining: score chunks live in a 3-deep [128,1024] PSUM pool shared with
the projection/out-proj accumulators (PSUM = 6+2 banks); single-head PV
(beta) emission lags alpha by ~3 head-pair groups and is interleaved between
alphas so PV matmuls pad the PE queue while QK waits on an sc slot; pc/pt
staging is 3-deep.  P transposes issue from the Sync queue only (ACT-issued
xbar transposes returned NaN on HW); their descriptor-gen is volume-bound
(~4.8us/MB) and they serialize, which with the softmax engine passes
(DVE max ~0.7ns/col, ACT exp ~1ns/col) sets the current ~570us pace.
"""

import numpy as np

import concourse.bacc as bacc
import concourse.tile as tile
from concourse import mybir
from concourse.bass_utils import run_bass_kernel_spmd

B, S, D = 4, 2048, 1024
H, DK = 16, 64
HL = 8            # heads per core
DL = HL * DK      # 512 local channels
N_CORES = 8
P = 128           # partitions
KT = D // P       # 8 contraction tiles
QT = S // P       # 16 q tiles
MS = 256          # proj m-slab (seq cols per stage)
MT = S // MS      # 8 slabs
NT = DL // P      # 4 channel slabs of 128
CHUNK = 1024      # score chunk (2 PSUM banks)
NEG = -60000.0    # causal mask additive (f16-representable)

f32 = mybir.dt.float32
f32r = mybir.dt.float32r
f16 = mybir.dt.float16
ALU = mybir.AluOpType
AF = mybir.ActivationFunctionType
AX = mybir.AxisListType.X

_cache = {}


def _build():
    nc = bacc.Bacc("TRN2", target_bir_lowering=False)

    def din(name, shape, dt):
        return nc.dram_tensor(name, shape, dt, kind="ExternalInput").ap()

    xq = din("xq", [D, S], f32r)
    xk = din("xk", [D, S], f32r)
    xv = din("xv", [D, S], f16)
    wq = din("wq", [D, DL], f32r)
    wk = din("wk", [D, DL], f32r)
    wv = din("wv", [D, DL], f16)
    wo = din("wo", [DL, D], f16)
    ident = din("ident", [P, P], f16)
    maskw = din("maskw", [P, 512], f16)
    y = nc.dram_tensor("y", [S, D], f16, kind="ExternalOutput").ap()

    with tile.TileContext(nc) as tc:
        _body(nc, tc, xq, xk, xv, wq, wk, wv, wo, ident, maskw, y)
    nc.compile()
    return nc


def _body(nc, tc, xq, xk, xv, wq, wk, wv, wo, ident, maskw, y):
    from contextlib import ExitStack
    ctx = ExitStack()
    with ctx:
        # ---------- long-lived tiles ----------
        persist = ctx.enter_context(tc.tile_pool(name="persist", bufs=1))
        # qsb[n]/ksb[n]: heads 2n (p0:64) and 2n+1 (p64:128), channel-major;
        # q pre-scaled by 1/8 so the QK^T PSUM holds final scaled scores.
        qsb = [persist.tile([P, S], f16, tag=f"qs_{n}", name=f"qs_{n}") for n in range(NT)]
        ksb = [persist.tile([P, S], f16, tag=f"ks_{n}", name=f"ks_{n}") for n in range(NT)]
        # vsb: [kpos, head, dk+1]; last column is ones so PV also yields Z
        vsb = [persist.tile([P, HL, DK + 1], f16, tag=f"v_{m}", name=f"v_{m}")
               for m in range(QT)]
        wq_sb = persist.tile([P, KT, DL], f32r, tag="wq", name="wq_sb")
        wk_sb = persist.tile([P, KT, DL], f32r, tag="wk", name="wk_sb")
        wv_sb = persist.tile([P, KT, DL], f16, tag="wv", name="wv_sb")
        wo_sb = persist.tile([P, NT, D], f16, tag="wo", name="wo_sb")
        ident_sb = persist.tile([P, P], f16, tag="ident")
        maskw_sb = persist.tile([P, 512], f16, tag="maskw")
        # weight/const loads: q first (q-proj is first consumer)
        for kh in range(2):
            ks = slice(kh * KT // 2, (kh + 1) * KT // 2)
            nc.sync.dma_start(out=wq_sb[:, ks], in_=wq.rearrange("(k p) n -> p k n", p=P)[:, ks])
        for kh in range(2):
            ks = slice(kh * KT // 2, (kh + 1) * KT // 2)
            nc.sync.dma_start(out=wk_sb[:, ks], in_=wk.rearrange("(k p) n -> p k n", p=P)[:, ks])
        nc.sync.dma_start(out=wv_sb, in_=wv.rearrange("(k p) n -> p k n", p=P))
        nc.sync.dma_start(out=wo_sb, in_=wo.rearrange("(j p) n -> p j n", p=P))
        nc.sync.dma_start(out=ident_sb, in_=ident)
        nc.sync.dma_start(out=maskw_sb, in_=maskw)
        for m in range(QT):
            nc.gpsimd.memset(vsb[m][:, :, DK:DK + 1], 1.0)

        xpool = ctx.enter_context(tc.tile_pool(name="xpool", bufs=2))
        scpool = ctx.enter_context(tc.tile_pool(name="scpool", bufs=3, space="PSUM"))
        pvpool = ctx.enter_context(tc.tile_pool(name="pvpool", bufs=2, space="PSUM"))
        pcpool = ctx.enter_context(tc.tile_pool(name="pcpool", bufs=3))
        ptpool = ctx.enter_context(tc.tile_pool(name="ptpool", bufs=4))
        stat = ctx.enter_context(tc.tile_pool(name="stat", bufs=8))
        ostage = ctx.enter_context(tc.tile_pool(name="ostage", bufs=2))
        otpool = ctx.enter_context(tc.tile_pool(name="otpool", bufs=3))
        ypool = ctx.enter_context(tc.tile_pool(name="ypool", bufs=1))

        KH = KT // 2

        def proj_slab(m):
            """Project q/k/v for seq positions [m*MS, (m+1)*MS)."""
            msl = slice(m * MS, (m + 1) * MS)
            # quarter-slab staging: 0.5MB DMAs keep x bursts short so the
            # serialized P-transpose packets are not starved on the shared
            # SDMA engines, and proj chains start on the first quarter
            xq_h = [xpool.tile([P, 2, MS], f32r, tag=f"xq{qtr}", name=f"xq_{qtr}")
                    for qtr in range(4)]
            xk_h = [xpool.tile([P, 2, MS], f32r, tag=f"xk{qtr}", name=f"xk_{qtr}")
                    for qtr in range(4)]
            for qtr in range(4):
                ksl = slice(qtr * 2, (qtr + 1) * 2)
                nc.gpsimd.dma_start(
                    out=xq_h[qtr], in_=xq.rearrange("(k p) s -> p k s", p=P)[:, ksl, msl])
                nc.gpsimd.dma_start(
                    out=xk_h[qtr], in_=xk.rearrange("(k p) s -> p k s", p=P)[:, ksl, msl])
            for n in range(NT):
                csl = slice(n * P, (n + 1) * P)
                psq = scpool.tile([P, CHUNK], f32, tag="sc", name="psq")
                for k in range(KT):
                    nc.tensor.matmul(psq[:, 0:MS], wq_sb[:, k, csl],
                                     xq_h[k // 2][:, k % 2],
                                     start=(k == 0), stop=(k == KT - 1))
                # pre-scale q by 1/8 into f16
                nc.scalar.activation(qsb[n][:, msl], psq[:, 0:MS], AF.Copy, scale=0.125)
                pop_beta()
                psk = scpool.tile([P, CHUNK], f32, tag="sc", name="psk")
                for k in range(KT):
                    nc.tensor.matmul(psk[:, 0:MS], wk_sb[:, k, csl],
                                     xk_h[k // 2][:, k % 2],
                                     start=(k == 0), stop=(k == KT - 1))
                nc.scalar.copy(ksb[n][:, msl], psk[:, 0:MS])
                pop_beta()
            # V projection -> seq-major [S, (h, dk)], fp16
            for half in range(2):
                kb = 2 * m + half
                xvt = xpool.tile([P, KT, P], f16, tag="xv", name="xvt")
                nc.gpsimd.dma_start(
                    out=xvt, in_=xv.rearrange("(k p) s -> p k s", p=P)[:, :, kb * P:(kb + 1) * P])
                psv = scpool.tile([P, CHUNK], f32, tag="sc", name="psv")
                for k in range(KT):
                    nc.tensor.matmul(psv[:, 0:512], xvt[:, k], wv_sb[:, k],
                                     start=(k == 0), stop=(k == KT - 1))
                nc.scalar.copy(vsb[kb][:, :, 0:DK],
                               psv[:, 0:512].rearrange("p (h d) -> p h d", h=HL))

        # ---------- attention ----------
        gstate = {}   # (qt, g) -> group state
        qstate = {}   # qt -> ostg
        otstate = {}  # qt -> transposed out tile

        def chunks_of(qt):
            klen = (qt + 1) * P
            return [(c0, min(c0 + CHUNK, klen)) for c0 in range(0, klen, CHUNK)]

        def alpha(qt, h):
            """QK^T + row-max + exp for one head into the group's pc2."""
            klen = (qt + 1) * P
            n, hp = h // 2, h % 2
            psl = slice(hp * DK, (hp + 1) * DK)
            g = h // 2  # head-pair group index == n
            st = gstate[(qt, g)]
            qtl = qsb[n][psl, qt * P:(qt + 1) * P]
            chunks = chunks_of(qt)
            mt = stat.tile([P, 4], f32, tag="mt", name="mt")
            scs = []
            for ci, (c0, c1) in enumerate(chunks):
                cl = c1 - c0
                sc = scpool.tile([P, CHUNK], f32, tag="sc", name="sc")
                scs.append(sc)
                if c1 == klen:
                    # final chunk: causal mask first — it reads only resident
                    # ident/maskw, so the PE starts the chunk the moment the
                    # psum slot frees; the k-dependent QK accumulates onto it
                    last = (cl - 1) // 512 * 512
                    nn = cl - last
                    nc.tensor.matmul(sc[:, last:cl], ident_sb[:],
                                     maskw_sb[:, 512 - nn:512],
                                     start=True, stop=False)
                    for n0 in range(0, last, 512):
                        nc.tensor.matmul(sc[:, n0:n0 + 512], qtl,
                                         ksb[n][psl, c0 + n0:c0 + n0 + 512],
                                         start=True, stop=True)
                    nc.tensor.matmul(sc[:, last:cl], qtl,
                                     ksb[n][psl, c0 + last:c0 + cl],
                                     start=False, stop=True)
                else:
                    for n0 in range(0, cl, 512):
                        nc.tensor.matmul(sc[:, n0:n0 + 512], qtl,
                                         ksb[n][psl, c0 + n0:c0 + n0 + 512],
                                         start=True, stop=True)
                if len(chunks) == 1:
                    nc.vector.reduce_max(mt[:, 0:1], sc[:, 0:cl], axis=AX, negate=True)
                else:
                    nc.vector.reduce_max(mt[:, ci:ci + 1], sc[:, 0:cl], axis=AX,
                                         negate=False)
            if len(chunks) == 1:
                mf = mt[:, 0:1]
            else:
                mf = stat.tile([P, 1], f32, tag="mf", name="mf")
                nc.vector.reduce_max(mf, mt[:, 0:len(chunks)], axis=AX, negate=True)
            for cj, (d0, d1) in enumerate(chunks):
                nc.scalar.activation(st["pc"][:, h % 2, d0:d1], scs[cj][:, 0:d1 - d0],
                                     AF.Exp, bias=mf, scale=1.0)

        def group_transpose(qt, g):
            st = gstate[(qt, g)]
            nc.sync.dma_start_transpose(st["pt"][:], st["pc"][:])

        def beta(qt, h):
            """PV + normalize for one head."""
            st = gstate[(qt, h // 2)]
            pt = st["pt"]
            hh = h % 2
            nkb = qt + 1
            if qt not in qstate:
                qstate[qt] = ostage.tile([P, DL], f16, tag="ostg", name="ostg")
            ostg = qstate[qt]
            ops = pvpool.tile([P, DK + 1], f32, tag="pv", name="ops")
            for kb in range(nkb):
                nc.tensor.matmul(ops[:], pt[:, hh * nkb + kb, :], vsb[kb][:, h, :],
                                 start=(kb == 0), stop=(kb == nkb - 1))
            rh = stat.tile([P, 1], f32, tag="rh", name="rh")
            nc.vector.reciprocal(rh, ops[:, DK:DK + 1])
            nc.vector.tensor_scalar(ostg[:, h * DK:(h + 1) * DK], ops[:, 0:DK],
                                    rh, None, op0=ALU.mult)

        def finish_qt(qt):
            ot = otpool.tile([P, NT, P], f16, tag="ot", name="ot")
            otstate[qt] = ot
            nc.sync.dma_start_transpose(ot[:], qstate[qt][:])
            for g in range(NT):
                del gstate[(qt, g)]

        def out_proj(qt):
            ysb = ypool.tile([P, D], f16, tag="y", name="ysb")
            yt = scpool.tile([P, CHUNK], f32, tag="sc", name="yt")
            for nn2 in range(2):
                for j in range(NT):
                    nc.tensor.matmul(
                        yt[:, nn2 * 512:(nn2 + 1) * 512],
                        otstate[qt][:, j, :],
                        wo_sb[:, j, nn2 * 512:(nn2 + 1) * 512],
                        start=(j == 0), stop=(j == NT - 1))
            nc.scalar.copy(ysb[:], yt[:])
            nc.gpsimd.dma_start(out=y[qt * P:(qt + 1) * P, :], in_=ysb[:])

        # ---------- interleaved emission ----------
        # beta emission lags alpha by ~3 groups (pc/pt bufs=3); single-head
        # betas are emitted BETWEEN alphas so their PV matmuls pad the PE
        # queue while the next alpha's QK waits on an sc-psum slot.
        THRESH = 6
        betaq = []   # (qt, h) heads whose transpose is emitted, beta pending
        bidx = 0

        def pop_beta():
            nonlocal bidx
            if len(betaq) - bidx > THRESH:
                bqt, bh = betaq[bidx]
                bidx += 1
                beta(bqt, bh)
                if bh == HL - 1:
                    finish_qt(bqt)
                    if bqt >= 1:
                        out_proj(bqt - 1)

        for m in range(MT):
            proj_slab(m)
            for qt in (2 * m, 2 * m + 1):
                for g in range(NT):
                    klen = (qt + 1) * P
                    pc = pcpool.tile([P, 2, klen], f16, tag="pc", name="pc")
                    pt = ptpool.tile([P, 2 * (qt + 1), P], f16, tag="pt", name="pt")
                    gstate[(qt, g)] = {"pc": pc, "pt": pt}
                    alpha(qt, 2 * g)
                    pop_beta()
                    alpha(qt, 2 * g + 1)
                    pop_beta()
                    group_transpose(qt, g)
                    betaq.append((qt, 2 * g))
                    betaq.append((qt, 2 * g + 1))
        while bidx < len(betaq):
            bqt, bh = betaq[bidx]
            bidx += 1
            beta(bqt, bh)
            if bh == HL - 1:
                finish_qt(bqt)
                if bqt >= 1:
                    out_proj(bqt - 1)
        out_proj(QT - 1)


def _host_prep(q, k, v, wq, wk, wv, wo):
    """Build the 8 per-core input maps."""
    ident = np.eye(P, dtype=np.float16)
    maskw = np.zeros((P, 512), np.float16)
    maskw[:, 384:512] = np.triu(np.full((P, P), NEG, np.float32), k=1).astype(np.float16)
    per_b = {}
    for b in range(B):
        per_b[b] = (
            np.ascontiguousarray(q[b].T.astype(np.float32)),
            np.ascontiguousarray(k[b].T.astype(np.float32)),
            np.ascontiguousarray(v[b].T.astype(np.float32)).astype(np.float16),
        )
    per_g = {}
    for g in range(2):
        cs = slice(g * DL, (g + 1) * DL)
        per_g[g] = (
            np.ascontiguousarray(wq[:, cs].astype(np.float32)),
            np.ascontiguousarray(wk[:, cs].astype(np.float32)),
            np.ascontiguousarray(wv[:, cs]).astype(np.float16),
            np.ascontiguousarray(wo[cs, :]).astype(np.float16),
        )
    in_maps = []
    for c in range(N_CORES):
        b, g = c // 2, c % 2
        xq_c, xk_c, xv_c = per_b[b]
        wq_c, wk_c, wv_c, wo_c = per_g[g]
        in_maps.append({
            "xq": xq_c, "xk": xk_c, "xv": xv_c,
            "wq": wq_c, "wk": wk_c, "wv": wv_c, "wo": wo_c,
            "ident": ident, "maskw": maskw,
        })
    return in_maps


def kernel(q, k, v, wq, wk, wv, wo):
    if "nc" not in _cache:
        _cache["nc"] = _build()
    nc = _cache["nc"]
    in_maps = _host_prep(np.asarray(q), np.asarray(k), np.asarray(v),
                         np.asarray(wq), np.asarray(wk), np.asarray(wv),
                         np.asarray(wo))
    res = run_bass_kernel_spmd(nc, in_maps, list(range(N_CORES)))
    out = np.empty((B, S, D), np.float32)
    for b in range(B):
        out[b] = res.results[2 * b]["y"].astype(np.float32) \
            + res.results[2 * b + 1]["y"].astype(np.float32)
    return out


if __name__ == "__main__":
    d = np.load("/root/problem/inputs_cache.npz")
    out = kernel(d["q"], d["k"], d["v"], d["wq"], d["wk"], d["wv"], d["wo"])
    ref = d["ref"]
    rel = np.linalg.norm(out - ref) / np.linalg.norm(ref)
    print(f"Relative error: {rel:.4e}")


# revision 27
# speedup vs baseline: 1.0232x; 1.0232x over previous
"""Causal multi-head attention (B=4, S=2048, D=1024, H=16) on 8 TRN2 cores.

Sharding: core c -> (batch b = c//2, head-group g = c%2, 8 heads each).
Host pre-transposes/splits inputs; device returns per-core partial outputs
y_c = attn_heads(g) @ wo[g-rows] in f16; host sums the two partials per batch.

Single interleaved phase: for each 256-seq-position slab, project q/k/v for
those positions (f32r matmuls, f16 outputs; q pre-scaled by 1/8), then run
attention for the two 128-row q-tiles the slab enables.  QK^T runs as f16
K=64 matmuls straight off the channel-major q/k tiles (measured: K=64 is
full rate, so no k duplication needed).  The causal mask (-60000) folds into
the score PSUM accumulation via an ident@maskw matmul.  Row-max on DVE,
exp on ScalarE (f16 out), P transposed to k-major via xbar DMA batched per
head pair, PV as N=65 f16 matmuls (ones column gives Z), normalize on DVE,
per-qt output projection pipelined one q-tile behind.

Pipelining: score chunks live in a 3-deep [128,1024] PSUM pool shared with
the projection/out-proj accumulators (PSUM = 6+2 banks); single-head PV
(beta) emission lags alpha by ~3 head-pair groups and is interleaved between
alphas so PV matmuls pad the PE queue while QK waits on an sc slot; pc/pt
staging is 3-deep.  P transposes issue from the Sync queue only (ACT-issued
xbar transposes returned NaN on HW); their descriptor-gen is volume-bound
(~4.8us/MB) and they serialize, which with the softmax engine passes
(DVE max ~0.7ns/col, ACT exp ~1ns/col) sets the current ~570us pace.
"""

import numpy as np

import concourse.bacc as bacc
import concourse.tile as tile
from concourse import mybir
from concourse.bass_utils import run_bass_kernel_spmd

B, S, D = 4, 2048, 1024
H, DK = 16, 64
HL = 8            # heads per core
DL = HL * DK      # 512 local channels
N_CORES = 8
P = 128           # partitions
KT = D // P       # 8 contraction tiles
QT = S // P       # 16 q tiles
MS = 256          # proj m-slab (seq cols per stage)
MT = S // MS      # 8 slabs
NT = DL // P      # 4 channel slabs of 128
CHUNK = 1024      # score chunk (2 PSUM banks)
NEG = -60000.0    # causal mask additive (f16-representable)

f32 = mybir.dt.float32
f32r = mybir.dt.float32r
f16 = mybir.dt.float16
ALU = mybir.AluOpType
AF = mybir.ActivationFunctionType
AX = mybir.AxisListType.X

_cache = {}


def _build():
    nc = bacc.Bacc("TRN2", target_bir_lowering=False)

    def din(name, shape, dt):
        return nc.dram_tensor(name, shape, dt, kind="ExternalInput").ap()

    xq = din("xq", [D, S], f32r)
    xk = din("xk", [D, S], f32r)
    xv = din("xv", [D, S], f16)
    wq = din("wq", [D, DL], f32r)
    wk = din("wk", [D, DL], f32r)
    wv = din("wv", [D, DL], f16)
    wo = din("wo", [DL, D], f16)
    ident = din("ident", [P, P], f16)
    maskw = din("maskw", [P, 512], f16)
    y = nc.dram_tensor("y", [S, D], f16, kind="ExternalOutput").ap()

    with tile.TileContext(nc) as tc:
        _body(nc, tc, xq, xk, xv, wq, wk, wv, wo, ident, maskw, y)
    nc.compile()
    return nc


def _body(nc, tc, xq, xk, xv, wq, wk, wv, wo, ident, maskw, y):
    from contextlib import ExitStack
    ctx = ExitStack()
    with ctx:
        # ---------- long-lived tiles ----------
        persist = ctx.enter_context(tc.tile_pool(name="persist", bufs=1))
        # qsb[n]/ksb[n]: heads 2n (p0:64) and 2n+1 (p64:128), channel-major;
        # q pre-scaled by 1/8 so the QK^T PSUM holds final scaled scores.
        qsb = [persist.tile([P, S], f16, tag=f"qs_{n}", name=f"qs_{n}") for n in range(NT)]
        ksb = [persist.tile([P, S], f16, tag=f"ks_{n}", name=f"ks_{n}") for n in range(NT)]
        # vsb: [kpos, head, dk+1]; last column is ones so PV also yields Z
        vsb = [persist.tile([P, HL, DK + 1], f16, tag=f"v_{m}", name=f"v_{m}")
               for m in range(QT)]
        wq_sb = persist.tile([P, KT, DL], f32r, tag="wq", name="wq_sb")
        wk_sb = persist.tile([P, KT, DL], f32r, tag="wk", name="wk_sb")
        wv_sb = persist.tile([P, KT, DL], f16, tag="wv", name="wv_sb")
        wo_sb = persist.tile([P, NT, D], f16, tag="wo", name="wo_sb")
        ident_sb = persist.tile([P, P], f16, tag="ident")
        maskw_sb = persist.tile([P, 512], f16, tag="maskw")
        # weight/const loads: q first (q-proj is first consumer)
        for kh in range(2):
            ks = slice(kh * KT // 2, (kh + 1) * KT // 2)
            nc.sync.dma_start(out=wq_sb[:, ks], in_=wq.rearrange("(k p) n -> p k n", p=P)[:, ks])
        for kh in range(2):
            ks = slice(kh * KT // 2, (kh + 1) * KT // 2)
            nc.sync.dma_start(out=wk_sb[:, ks], in_=wk.rearrange("(k p) n -> p k n", p=P)[:, ks])
        nc.sync.dma_start(out=wv_sb, in_=wv.rearrange("(k p) n -> p k n", p=P))
        nc.sync.dma_start(out=wo_sb, in_=wo.rearrange("(j p) n -> p j n", p=P))
        nc.sync.dma_start(out=ident_sb, in_=ident)
        nc.sync.dma_start(out=maskw_sb, in_=maskw)
        for m in range(QT):
            nc.gpsimd.memset(vsb[m][:, :, DK:DK + 1], 1.0)

        xpool = ctx.enter_context(tc.tile_pool(name="xpool", bufs=2))
        scpool = ctx.enter_context(tc.tile_pool(name="scpool", bufs=3, space="PSUM"))
        pvpool = ctx.enter_context(tc.tile_pool(name="pvpool", bufs=2, space="PSUM"))
        pcpool = ctx.enter_context(tc.tile_pool(name="pcpool", bufs=3))
        ptpool = ctx.enter_context(tc.tile_pool(name="ptpool", bufs=4))
        stat = ctx.enter_context(tc.tile_pool(name="stat", bufs=8))
        ostage = ctx.enter_context(tc.tile_pool(name="ostage", bufs=2))
        otpool = ctx.enter_context(tc.tile_pool(name="otpool", bufs=3))
        ypool = ctx.enter_context(tc.tile_pool(name="ypool", bufs=1))

        KH = KT // 2

        def proj_slab(m):
            """Project q/k/v for seq positions [m*MS, (m+1)*MS)."""
            msl = slice(m * MS, (m + 1) * MS)
            xq_h = [xpool.tile([P, KH, MS], f32r, tag=f"xq{half}", name=f"xq_{half}")
                    for half in range(2)]
            xk_h = [xpool.tile([P, KH, MS], f32r, tag=f"xk{half}", name=f"xk_{half}")
                    for half in range(2)]
            for half in range(2):
                ksl = slice(half * KH, (half + 1) * KH)
                nc.gpsimd.dma_start(
                    out=xq_h[half], in_=xq.rearrange("(k p) s -> p k s", p=P)[:, ksl, msl])
                nc.gpsimd.dma_start(
                    out=xk_h[half], in_=xk.rearrange("(k p) s -> p k s", p=P)[:, ksl, msl])
            for n in range(NT):
                csl = slice(n * P, (n + 1) * P)
                psq = scpool.tile([P, CHUNK], f32, tag="sc", name="psq")
                for k in range(KT):
                    nc.tensor.matmul(psq[:, 0:MS], wq_sb[:, k, csl],
                                     xq_h[k // KH][:, k % KH],
                                     start=(k == 0), stop=(k == KT - 1))
                # pre-scale q by 1/8 into f16
                nc.scalar.activation(qsb[n][:, msl], psq[:, 0:MS], AF.Copy, scale=0.125)
                pop_beta()
                psk = scpool.tile([P, CHUNK], f32, tag="sc", name="psk")
                for k in range(KT):
                    nc.tensor.matmul(psk[:, 0:MS], wk_sb[:, k, csl],
                                     xk_h[k // KH][:, k % KH],
                                     start=(k == 0), stop=(k == KT - 1))
                nc.scalar.copy(ksb[n][:, msl], psk[:, 0:MS])
                pop_beta()
            # V projection -> seq-major [S, (h, dk)], fp16
            for half in range(2):
                kb = 2 * m + half
                xvt = xpool.tile([P, KT, P], f16, tag="xv", name="xvt")
                nc.gpsimd.dma_start(
                    out=xvt, in_=xv.rearrange("(k p) s -> p k s", p=P)[:, :, kb * P:(kb + 1) * P])
                psv = scpool.tile([P, CHUNK], f32, tag="sc", name="psv")
                for k in range(KT):
                    nc.tensor.matmul(psv[:, 0:512], xvt[:, k], wv_sb[:, k],
                                     start=(k == 0), stop=(k == KT - 1))
                nc.scalar.copy(vsb[kb][:, :, 0:DK],
                               psv[:, 0:512].rearrange("p (h d) -> p h d", h=HL))

        # ---------- attention ----------
        gstate = {}   # (qt, g) -> group state
        qstate = {}   # qt -> ostg
        otstate = {}  # qt -> transposed out tile

        def chunks_of(qt):
            klen = (qt + 1) * P
            return [(c0, min(c0 + CHUNK, klen)) for c0 in range(0, klen, CHUNK)]

        def alpha(qt, h):
            """QK^T + row-max + exp for one head into the group's pc2."""
            klen = (qt + 1) * P
            n, hp = h // 2, h % 2
            psl = slice(hp * DK, (hp + 1) * DK)
            g = h // 2  # head-pair group index == n
            st = gstate[(qt, g)]
            qtl = qsb[n][psl, qt * P:(qt + 1) * P]
            chunks = chunks_of(qt)
            mt = stat.tile([P, 4], f32, tag="mt", name="mt")
            scs = []
            for ci, (c0, c1) in enumerate(chunks):
                cl = c1 - c0
                sc = scpool.tile([P, CHUNK], f32, tag="sc", name="sc")
                scs.append(sc)
                if c1 == klen:
                    # final chunk: causal mask first — it reads only resident
                    # ident/maskw, so the PE starts the chunk the moment the
                    # psum slot frees; the k-dependent QK accumulates onto it
                    last = (cl - 1) // 512 * 512
                    nn = cl - last
                    nc.tensor.matmul(sc[:, last:cl], ident_sb[:],
                                     maskw_sb[:, 512 - nn:512],
                                     start=True, stop=False)
                    for n0 in range(0, last, 512):
                        nc.tensor.matmul(sc[:, n0:n0 + 512], qtl,
                                         ksb[n][psl, c0 + n0:c0 + n0 + 512],
                                         start=True, stop=True)
                    nc.tensor.matmul(sc[:, last:cl], qtl,
                                     ksb[n][psl, c0 + last:c0 + cl],
                                     start=False, stop=True)
                else:
                    for n0 in range(0, cl, 512):
                        nc.tensor.matmul(sc[:, n0:n0 + 512], qtl,
                                         ksb[n][psl, c0 + n0:c0 + n0 + 512],
                                         start=True, stop=True)
                if len(chunks) == 1:
                    nc.vector.reduce_max(mt[:, 0:1], sc[:, 0:cl], axis=AX, negate=True)
                else:
                    nc.vector.reduce_max(mt[:, ci:ci + 1], sc[:, 0:cl], axis=AX,
                                         negate=False)
            if len(chunks) == 1:
                mf = mt[:, 0:1]
            else:
                mf = stat.tile([P, 1], f32, tag="mf", name="mf")
                nc.vector.reduce_max(mf, mt[:, 0:len(chunks)], axis=AX, negate=True)
            for cj, (d0, d1) in enumerate(chunks):
                nc.scalar.activation(st["pc"][:, h % 2, d0:d1], scs[cj][:, 0:d1 - d0],
                                     AF.Exp, bias=mf, scale=1.0)

        def group_transpose(qt, g):
            st = gstate[(qt, g)]
            nc.sync.dma_start_transpose(st["pt"][:], st["pc"][:])

        def beta(qt, h):
            """PV + normalize for one head."""
            st = gstate[(qt, h // 2)]
            pt = st["pt"]
            hh = h % 2
            nkb = qt + 1
            if qt not in qstate:
                qstate[qt] = ostage.tile([P, DL], f16, tag="ostg", name="ostg")
            ostg = qstate[qt]
            ops = pvpool.tile([P, DK + 1], f32, tag="pv", name="ops")
            for kb in range(nkb):
                nc.tensor.matmul(ops[:], pt[:, hh * nkb + kb, :], vsb[kb][:, h, :],
                                 start=(kb == 0), stop=(kb == nkb - 1))
            rh = stat.tile([P, 1], f32, tag="rh", name="rh")
            nc.vector.reciprocal(rh, ops[:, DK:DK + 1])
            nc.vector.tensor_scalar(ostg[:, h * DK:(h + 1) * DK], ops[:, 0:DK],
                                    rh, None, op0=ALU.mult)

        def finish_qt(qt):
            ot = otpool.tile([P, NT, P], f16, tag="ot", name="ot")
            otstate[qt] = ot
            nc.sync.dma_start_transpose(ot[:], qstate[qt][:])
            for g in range(NT):
                del gstate[(qt, g)]

        def out_proj(qt):
            ysb = ypool.tile([P, D], f16, tag="y", name="ysb")
            yt = scpool.tile([P, CHUNK], f32, tag="sc", name="yt")
            for nn2 in range(2):
                for j in range(NT):
                    nc.tensor.matmul(
                        yt[:, nn2 * 512:(nn2 + 1) * 512],
                        otstate[qt][:, j, :],
                        wo_sb[:, j, nn2 * 512:(nn2 + 1) * 512],
                        start=(j == 0), stop=(j == NT - 1))
            nc.scalar.copy(ysb[:], yt[:])
            nc.gpsimd.dma_start(out=y[qt * P:(qt + 1) * P, :], in_=ysb[:])

        # ---------- interleaved emission ----------
        # beta emission lags alpha by ~3 groups (pc/pt bufs=3); single-head
        # betas are emitted BETWEEN alphas so their PV matmuls pad the PE
        # queue while the next alpha's QK waits on an sc-psum slot.
        THRESH = 6
        betaq = []   # (qt, h) heads whose transpose is emitted, beta pending
        bidx = 0

        def pop_beta():
            nonlocal bidx
            if len(betaq) - bidx > THRESH:
                bqt, bh = betaq[bidx]
                bidx += 1
                beta(bqt, bh)
                if bh == HL - 1:
                    finish_qt(bqt)
                    if bqt >= 1:
                        out_proj(bqt - 1)

        for m in range(MT):
            proj_slab(m)
            for qt in (2 * m, 2 * m + 1):
                for g in range(NT):
                    klen = (qt + 1) * P
                    pc = pcpool.tile([P, 2, klen], f16, tag="pc", name="pc")
                    pt = ptpool.tile([P, 2 * (qt + 1), P], f16, tag="pt", name="pt")
                    gstate[(qt, g)] = {"pc": pc, "pt": pt}
                    alpha(qt, 2 * g)
                    pop_beta()
                    alpha(qt, 2 * g + 1)
                    pop_beta()
                    group_transpose(qt, g)
                    betaq.append((qt, 2 * g))
                    betaq.append((qt, 2 * g + 1))
        while bidx < len(betaq):
            bqt, bh = betaq[bidx]
            bidx += 1
            beta(bqt, bh)
            if bh == HL - 1:
                finish_qt(bqt)
                if bqt >= 1:
                    out_proj(bqt - 1)
        out_proj(QT - 1)


def _host_prep(q, k, v, wq, wk, wv, wo):
    """Build the 8 per-core input maps."""
    ident = np.eye(P, dtype=np.float16)
    maskw = np.zeros((P, 512), np.float16)
    maskw[:, 384:512] = np.triu(np.full((P, P), NEG, np.float32), k=1).astype(np.float16)
    per_b = {}
    for b in range(B):
        per_b[b] = (
            np.ascontiguousarray(q[b].T.astype(np.float32)),
            np.ascontiguousarray(k[b].T.astype(np.float32)),
            np.ascontiguousarray(v[b].T.astype(np.float32)).astype(np.float16),
        )
    per_g = {}
    for g in range(2):
        cs = slice(g * DL, (g + 1) * DL)
        per_g[g] = (
            np.ascontiguousarray(wq[:, cs].astype(np.float32)),
            np.ascontiguousarray(wk[:, cs].astype(np.float32)),
            np.ascontiguousarray(wv[:, cs]).astype(np.float16),
            np.ascontiguousarray(wo[cs, :]).astype(np.float16),
        )
    in_maps = []
    for c in range(N_CORES):
        b, g = c // 2, c % 2
        xq_c, xk_c, xv_c = per_b[b]
        wq_c, wk_c, wv_c, wo_c = per_g[g]
        in_maps.append({
            "xq": xq_c, "xk": xk_c, "xv": xv_c,
            "wq": wq_c, "wk": wk_c, "wv": wv_c, "wo": wo_c,
            "ident": ident, "maskw": maskw,
        })
    return in_maps


def kernel(q, k, v, wq, wk, wv, wo):
    if "nc" not in _cache:
        _cache["nc"] = _build()
    nc = _cache["nc"]
    in_maps = _host_prep(np.asarray(q), np.asarray(k), np.asarray(v),
                         np.asarray(wq), np.asarray(wk), np.asarray(wv),
                         np.asarray(wo))
    res = run_bass_kernel_spmd(nc, in_maps, list(range(N_CORES)))
    out = np.empty((B, S, D), np.float32)
    for b in range(B):
        out[b] = res.results[2 * b]["y"].astype(np.float32) \
            + res.results[2 * b + 1]["y"].astype(np.float32)
    return out


if __name__ == "__main__":
    d = np.load("/root/problem/inputs_cache.npz")
    out = kernel(d["q"], d["k"], d["v"], d["wq"], d["wk"], d["wv"], d["wo"])
    ref = d["ref"]
    rel = np.linalg.norm(out - ref) / np.linalg.norm(ref)
    print(f"Relative error: {rel:.4e}")


# revision 28
# speedup vs baseline: 1.0376x; 1.0141x over previous
"""Causal multi-head attention (B=4, S=2048, D=1024, H=16) on 8 TRN2 cores.

Sharding: core c -> (batch b = c//2, head-group g = c%2, 8 heads each).
Host pre-transposes/splits inputs; device returns per-core partial outputs
y_c = attn_heads(g) @ wo[g-rows] in f16; host sums the two partials per batch.

Single interleaved phase: for each 256-seq-position slab, project q/k/v for
those positions (f32r matmuls, f16 outputs; q pre-scaled by 1/8), then run
attention for the two 128-row q-tiles the slab enables.  QK^T runs as f16
K=64 matmuls straight off the channel-major q/k tiles (measured: K=64 is
full rate, so no k duplication needed).  The causal mask (-60000) folds into
the score PSUM accumulation via an ident@maskw matmul.  Row-max on DVE,
exp on ScalarE (f16 out), P transposed to k-major via xbar DMA batched per
head pair, PV as N=65 f16 matmuls (ones column gives Z), normalize on DVE,
per-qt output projection pipelined one q-tile behind.

Pipelining: score chunks live in a 3-deep [128,1024] PSUM pool shared with
the projection/out-proj accumulators (PSUM = 6+2 banks); single-head PV
(beta) emission lags alpha by ~3 head-pair groups and is interleaved between
alphas so PV matmuls pad the PE queue while QK waits on an sc slot; pc/pt
staging is 3-deep.  P transposes issue from the Sync queue only (ACT-issued
xbar transposes returned NaN on HW); their descriptor-gen is volume-bound
(~4.8us/MB) and they serialize, which with the softmax engine passes
(DVE max ~0.7ns/col, ACT exp ~1ns/col) sets the current ~570us pace.
"""

import numpy as np

import concourse.bacc as bacc
import concourse.tile as tile
from concourse import mybir
from concourse.bass_utils import run_bass_kernel_spmd

B, S, D = 4, 2048, 1024
H, DK = 16, 64
HL = 8            # heads per core
DL = HL * DK      # 512 local channels
N_CORES = 8
P = 128           # partitions
KT = D // P       # 8 contraction tiles
QT = S // P       # 16 q tiles
MS = 256          # proj m-slab (seq cols per stage)
MT = S // MS      # 8 slabs
NT = DL // P      # 4 channel slabs of 128
CHUNK = 1024      # score chunk (2 PSUM banks)
NEG = -60000.0    # causal mask additive (f16-representable)

f32 = mybir.dt.float32
f32r = mybir.dt.float32r
f16 = mybir.dt.float16
ALU = mybir.AluOpType
AF = mybir.ActivationFunctionType
AX = mybir.AxisListType.X

_cache = {}


def _build():
    nc = bacc.Bacc("TRN2", target_bir_lowering=False)

    def din(name, shape, dt):
        return nc.dram_tensor(name, shape, dt, kind="ExternalInput").ap()

    xq = din("xq", [D, S], f32r)
    xk = din("xk", [D, S], f32r)
    xv = din("xv", [D, S], f16)
    wq = din("wq", [D, DL], f32r)
    wk = din("wk", [D, DL], f32r)
    wv = din("wv", [D, DL], f16)
    wo = din("wo", [DL, D], f16)
    ident = din("ident", [P, P], f16)
    maskw = din("maskw", [P, 512], f16)
    y = nc.dram_tensor("y", [S, D], f16, kind="ExternalOutput").ap()

    with tile.TileContext(nc) as tc:
        _body(nc, tc, xq, xk, xv, wq, wk, wv, wo, ident, maskw, y)
    nc.compile()
    return nc


def _body(nc, tc, xq, xk, xv, wq, wk, wv, wo, ident, maskw, y):
    from contextlib import ExitStack
    ctx = ExitStack()
    with ctx:
        # ---------- long-lived tiles ----------
        persist = ctx.enter_context(tc.tile_pool(name="persist", bufs=1))
        # qsb[n]/ksb[n]: heads 2n (p0:64) and 2n+1 (p64:128), channel-major;
        # q pre-scaled by 1/8 so the QK^T PSUM holds final scaled scores.
        qsb = [persist.tile([P, S], f16, tag=f"qs_{n}", name=f"qs_{n}") for n in range(NT)]
        ksb = [persist.tile([P, S], f16, tag=f"ks_{n}", name=f"ks_{n}") for n in range(NT)]
        # vsb: [kpos, head, dk+1]; last column is ones so PV also yields Z
        vsb = [persist.tile([P, HL, DK + 1], f16, tag=f"v_{m}", name=f"v_{m}")
               for m in range(QT)]
        wq_sb = persist.tile([P, KT, DL], f32r, tag="wq", name="wq_sb")
        wk_sb = persist.tile([P, KT, DL], f32r, tag="wk", name="wk_sb")
        wv_sb = persist.tile([P, KT, DL], f16, tag="wv", name="wv_sb")
        wo_sb = persist.tile([P, NT, D], f16, tag="wo", name="wo_sb")
        ident_sb = persist.tile([P, P], f16, tag="ident")
        maskw_sb = persist.tile([P, 512], f16, tag="maskw")
        # weight/const loads: q first (q-proj is first consumer)
        for kh in range(2):
            ks = slice(kh * KT // 2, (kh + 1) * KT // 2)
            nc.sync.dma_start(out=wq_sb[:, ks], in_=wq.rearrange("(k p) n -> p k n", p=P)[:, ks])
        for kh in range(2):
            ks = slice(kh * KT // 2, (kh + 1) * KT // 2)
            nc.sync.dma_start(out=wk_sb[:, ks], in_=wk.rearrange("(k p) n -> p k n", p=P)[:, ks])
        nc.sync.dma_start(out=wv_sb, in_=wv.rearrange("(k p) n -> p k n", p=P))
        nc.sync.dma_start(out=wo_sb, in_=wo.rearrange("(j p) n -> p j n", p=P))
        nc.sync.dma_start(out=ident_sb, in_=ident)
        nc.sync.dma_start(out=maskw_sb, in_=maskw)
        for m in range(QT):
            nc.gpsimd.memset(vsb[m][:, :, DK:DK + 1], 1.0)

        xpool = ctx.enter_context(tc.tile_pool(name="xpool", bufs=2))
        scpool = ctx.enter_context(tc.tile_pool(name="scpool", bufs=3, space="PSUM"))
        pvpool = ctx.enter_context(tc.tile_pool(name="pvpool", bufs=2, space="PSUM"))
        pcpool = ctx.enter_context(tc.tile_pool(name="pcpool", bufs=4))
        ptpool = ctx.enter_context(tc.tile_pool(name="ptpool", bufs=4))
        stat = ctx.enter_context(tc.tile_pool(name="stat", bufs=8))
        ostage = ctx.enter_context(tc.tile_pool(name="ostage", bufs=2))
        otpool = ctx.enter_context(tc.tile_pool(name="otpool", bufs=3))
        ypool = ctx.enter_context(tc.tile_pool(name="ypool", bufs=1))

        KH = KT // 2

        def proj_slab(m):
            """Project q/k/v for seq positions [m*MS, (m+1)*MS)."""
            msl = slice(m * MS, (m + 1) * MS)
            xq_h = [xpool.tile([P, KH, MS], f32r, tag=f"xq{half}", name=f"xq_{half}")
                    for half in range(2)]
            xk_h = [xpool.tile([P, KH, MS], f32r, tag=f"xk{half}", name=f"xk_{half}")
                    for half in range(2)]
            for half in range(2):
                ksl = slice(half * KH, (half + 1) * KH)
                nc.gpsimd.dma_start(
                    out=xq_h[half], in_=xq.rearrange("(k p) s -> p k s", p=P)[:, ksl, msl])
                nc.gpsimd.dma_start(
                    out=xk_h[half], in_=xk.rearrange("(k p) s -> p k s", p=P)[:, ksl, msl])
            for n in range(NT):
                csl = slice(n * P, (n + 1) * P)
                psq = scpool.tile([P, CHUNK], f32, tag="sc", name="psq")
                for k in range(KT):
                    nc.tensor.matmul(psq[:, 0:MS], wq_sb[:, k, csl],
                                     xq_h[k // KH][:, k % KH],
                                     start=(k == 0), stop=(k == KT - 1))
                # pre-scale q by 1/8 into f16
                nc.scalar.activation(qsb[n][:, msl], psq[:, 0:MS], AF.Copy, scale=0.125)
                pop_beta()
                psk = scpool.tile([P, CHUNK], f32, tag="sc", name="psk")
                for k in range(KT):
                    nc.tensor.matmul(psk[:, 0:MS], wk_sb[:, k, csl],
                                     xk_h[k // KH][:, k % KH],
                                     start=(k == 0), stop=(k == KT - 1))
                nc.scalar.copy(ksb[n][:, msl], psk[:, 0:MS])
                pop_beta()
            # V projection -> seq-major [S, (h, dk)], fp16
            for half in range(2):
                kb = 2 * m + half
                xvt = xpool.tile([P, KT, P], f16, tag="xv", name="xvt")
                nc.gpsimd.dma_start(
                    out=xvt, in_=xv.rearrange("(k p) s -> p k s", p=P)[:, :, kb * P:(kb + 1) * P])
                psv = scpool.tile([P, CHUNK], f32, tag="sc", name="psv")
                for k in range(KT):
                    nc.tensor.matmul(psv[:, 0:512], xvt[:, k], wv_sb[:, k],
                                     start=(k == 0), stop=(k == KT - 1))
                nc.scalar.copy(vsb[kb][:, :, 0:DK],
                               psv[:, 0:512].rearrange("p (h d) -> p h d", h=HL))

        # ---------- attention ----------
        gstate = {}   # (qt, g) -> group state
        qstate = {}   # qt -> ostg
        otstate = {}  # qt -> transposed out tile

        def chunks_of(qt):
            klen = (qt + 1) * P
            return [(c0, min(c0 + CHUNK, klen)) for c0 in range(0, klen, CHUNK)]

        def alpha(qt, h):
            """QK^T + row-max + exp for one head into the group's pc2."""
            klen = (qt + 1) * P
            n, hp = h // 2, h % 2
            psl = slice(hp * DK, (hp + 1) * DK)
            g = h // 2  # head-pair group index == n
            st = gstate[(qt, g)]
            qtl = qsb[n][psl, qt * P:(qt + 1) * P]
            chunks = chunks_of(qt)
            mt = stat.tile([P, 4], f32, tag="mt", name="mt")
            scs = []
            for ci, (c0, c1) in enumerate(chunks):
                cl = c1 - c0
                sc = scpool.tile([P, CHUNK], f32, tag="sc", name="sc")
                scs.append(sc)
                if c1 == klen:
                    # final chunk: causal mask first — it reads only resident
                    # ident/maskw, so the PE starts the chunk the moment the
                    # psum slot frees; the k-dependent QK accumulates onto it
                    last = (cl - 1) // 512 * 512
                    nn = cl - last
                    nc.tensor.matmul(sc[:, last:cl], ident_sb[:],
                                     maskw_sb[:, 512 - nn:512],
                                     start=True, stop=False)
                    for n0 in range(0, last, 512):
                        nc.tensor.matmul(sc[:, n0:n0 + 512], qtl,
                                         ksb[n][psl, c0 + n0:c0 + n0 + 512],
                                         start=True, stop=True)
                    nc.tensor.matmul(sc[:, last:cl], qtl,
                                     ksb[n][psl, c0 + last:c0 + cl],
                                     start=False, stop=True)
                else:
                    for n0 in range(0, cl, 512):
                        nc.tensor.matmul(sc[:, n0:n0 + 512], qtl,
                                         ksb[n][psl, c0 + n0:c0 + n0 + 512],
                                         start=True, stop=True)
                if len(chunks) == 1:
                    nc.vector.reduce_max(mt[:, 0:1], sc[:, 0:cl], axis=AX, negate=True)
                else:
                    nc.vector.reduce_max(mt[:, ci:ci + 1], sc[:, 0:cl], axis=AX,
                                         negate=False)
            if len(chunks) == 1:
                mf = mt[:, 0:1]
            else:
                mf = stat.tile([P, 1], f32, tag="mf", name="mf")
                nc.vector.reduce_max(mf, mt[:, 0:len(chunks)], axis=AX, negate=True)
            for cj, (d0, d1) in enumerate(chunks):
                nc.scalar.activation(st["pc"][:, h % 2, d0:d1], scs[cj][:, 0:d1 - d0],
                                     AF.Exp, bias=mf, scale=1.0)

        def group_transpose(qt, g):
            st = gstate[(qt, g)]
            nc.sync.dma_start_transpose(st["pt"][:], st["pc"][:])

        def beta(qt, h):
            """PV + normalize for one head."""
            st = gstate[(qt, h // 2)]
            pt = st["pt"]
            hh = h % 2
            nkb = qt + 1
            if qt not in qstate:
                qstate[qt] = ostage.tile([P, DL], f16, tag="ostg", name="ostg")
            ostg = qstate[qt]
            ops = pvpool.tile([P, DK + 1], f32, tag="pv", name="ops")
            for kb in range(nkb):
                nc.tensor.matmul(ops[:], pt[:, hh * nkb + kb, :], vsb[kb][:, h, :],
                                 start=(kb == 0), stop=(kb == nkb - 1))
            rh = stat.tile([P, 1], f32, tag="rh", name="rh")
            nc.vector.reciprocal(rh, ops[:, DK:DK + 1])
            nc.vector.tensor_scalar(ostg[:, h * DK:(h + 1) * DK], ops[:, 0:DK],
                                    rh, None, op0=ALU.mult)

        def finish_qt(qt):
            ot = otpool.tile([P, NT, P], f16, tag="ot", name="ot")
            otstate[qt] = ot
            nc.sync.dma_start_transpose(ot[:], qstate[qt][:])
            for g in range(NT):
                del gstate[(qt, g)]

        def out_proj(qt):
            ysb = ypool.tile([P, D], f16, tag="y", name="ysb")
            yt = scpool.tile([P, CHUNK], f32, tag="sc", name="yt")
            for nn2 in range(2):
                for j in range(NT):
                    nc.tensor.matmul(
                        yt[:, nn2 * 512:(nn2 + 1) * 512],
                        otstate[qt][:, j, :],
                        wo_sb[:, j, nn2 * 512:(nn2 + 1) * 512],
                        start=(j == 0), stop=(j == NT - 1))
            nc.scalar.copy(ysb[:], yt[:])
            nc.gpsimd.dma_start(out=y[qt * P:(qt + 1) * P, :], in_=ysb[:])

        # ---------- interleaved emission ----------
        # beta emission lags alpha by ~3 groups (pc/pt bufs=3); single-head
        # betas are emitted BETWEEN alphas so their PV matmuls pad the PE
        # queue while the next alpha's QK waits on an sc-psum slot.
        THRESH = 6
        betaq = []   # (qt, h) heads whose transpose is emitted, beta pending
        bidx = 0

        def pop_beta():
            nonlocal bidx
            if len(betaq) - bidx > THRESH:
                bqt, bh = betaq[bidx]
                bidx += 1
                beta(bqt, bh)
                if bh == HL - 1:
                    finish_qt(bqt)
                    if bqt >= 1:
                        out_proj(bqt - 1)

        for m in range(MT):
            proj_slab(m)
            for qt in (2 * m, 2 * m + 1):
                for g in range(NT):
                    klen = (qt + 1) * P
                    pc = pcpool.tile([P, 2, klen], f16, tag="pc", name="pc")
                    pt = ptpool.tile([P, 2 * (qt + 1), P], f16, tag="pt", name="pt")
                    gstate[(qt, g)] = {"pc": pc, "pt": pt}
                    alpha(qt, 2 * g)
                    pop_beta()
                    alpha(qt, 2 * g + 1)
                    pop_beta()
                    group_transpose(qt, g)
                    betaq.append((qt, 2 * g))
                    betaq.append((qt, 2 * g + 1))
        while bidx < len(betaq):
            bqt, bh = betaq[bidx]
            bidx += 1
            beta(bqt, bh)
            if bh == HL - 1:
                finish_qt(bqt)
                if bqt >= 1:
                    out_proj(bqt - 1)
        out_proj(QT - 1)


def _host_prep(q, k, v, wq, wk, wv, wo):
    """Build the 8 per-core input maps."""
    ident = np.eye(P, dtype=np.float16)
    maskw = np.zeros((P, 512), np.float16)
    maskw[:, 384:512] = np.triu(np.full((P, P), NEG, np.float32), k=1).astype(np.float16)
    per_b = {}
    for b in range(B):
        per_b[b] = (
            np.ascontiguousarray(q[b].T.astype(np.float32)),
            np.ascontiguousarray(k[b].T.astype(np.float32)),
            np.ascontiguousarray(v[b].T.astype(np.float32)).astype(np.float16),
        )
    per_g = {}
    for g in range(2):
        cs = slice(g * DL, (g + 1) * DL)
        per_g[g] = (
            np.ascontiguousarray(wq[:, cs].astype(np.float32)),
            np.ascontiguousarray(wk[:, cs].astype(np.float32)),
            np.ascontiguousarray(wv[:, cs]).astype(np.float16),
            np.ascontiguousarray(wo[cs, :]).astype(np.float16),
        )
    in_maps = []
    for c in range(N_CORES):
        b, g = c // 2, c % 2
        xq_c, xk_c, xv_c = per_b[b]
        wq_c, wk_c, wv_c, wo_c = per_g[g]
        in_maps.append({
            "xq": xq_c, "xk": xk_c, "xv": xv_c,
            "wq": wq_c, "wk": wk_c, "wv": wv_c, "wo": wo_c,
            "ident": ident, "maskw": maskw,
        })
    return in_maps


def kernel(q, k, v, wq, wk, wv, wo):
    if "nc" not in _cache:
        _cache["nc"] = _build()
    nc = _cache["nc"]
    in_maps = _host_prep(np.asarray(q), np.asarray(k), np.asarray(v),
                         np.asarray(wq), np.asarray(wk), np.asarray(wv),
                         np.asarray(wo))
    res = run_bass_kernel_spmd(nc, in_maps, list(range(N_CORES)))
    out = np.empty((B, S, D), np.float32)
    for b in range(B):
        out[b] = res.results[2 * b]["y"].astype(np.float32) \
            + res.results[2 * b + 1]["y"].astype(np.float32)
    return out


if __name__ == "__main__":
    d = np.load("/root/problem/inputs_cache.npz")
    out = kernel(d["q"], d["k"], d["v"], d["wq"], d["wk"], d["wv"], d["wo"])
    ref = d["ref"]
    rel = np.linalg.norm(out - ref) / np.linalg.norm(ref)
    print(f"Relative error: {rel:.4e}")
